# revision 10
# baseline (speedup 1.0000x reference)
"""COO SpMM (out[b, dst] += w_e * x[b, src_e]) on 8 Trainium2 NeuronCores.

Shapes (hardcoded from the problem spec):
  x [32, 65536] f32, weights [1048576] f32, dst_idx/src_idx [1048576] i32
  out [32, 65536] f32

Strategy (SPMD over 8 cores, identical program, per-core data):
- Each core owns a dst shard of 8192 nodes (snake assignment by global dst
  degree, so per-core edge counts are balanced). No cross-core reduction.
- x.T is padded to [65536, 64] f32 rows (32 batch lanes + 32 pad = 256 B
  rows) and staged in HBM per core.
- Core edges are split by src half (int16 gather-index range); per
  (core, half) the local dst are sorted by half-degree descending; round r
  takes the r-th edge of every dst with >= r+1 edges in that half, so each
  round touches a prefix of the dst slots and each dst appears at most once
  per round (conflict-free accumulation).
- A global round schedule (max prefix length over all 16 (core, half)
  groups, padded to 128) keeps the device program identical across cores;
  padding slots gather row 0 with weight 0.
- Device per core: gpsimd.dma_gather streams 256 B x.T rows from HBM into
  SBUF [128, blk, 64] (stream pos i -> partition i%128, block i//128);
  DVE multiplies in-place by the per-row weight (stride-0 broadcast along
  the 32 batch lanes); round 0 is a copy and rounds r>=1 are adds into
  acc_h [128, 64, 32]; both half-accumulators are DMAed out.
- Host: per (core, half) un-permute the degree-sort, sum halves, place the
  shard columns into the full [32, 65536] output.
"""
import os

import numpy as np

import concourse.bacc as bacc
import concourse.mybir as mybir
from concourse.bass_utils import run_bass_kernel_spmd
from concourse.tile import TileContext

B = 32
IN_SIZE = 65536
OUT_SIZE = 65536
NCORES = 8
SHARD = OUT_SIZE // NCORES          # 8192
NHALF = 2
HALF = IN_SIZE // NHALF             # 32768
ELEM = 64                           # padded x.T row, f32 units (256 B)
RND_ALIGN = 128
CHUNK_IDX = 4096                    # idxs per dma_gather call
ACC_BLK = SHARD // 128              # 64

LAST_EXEC_NS = None                 # set when SPMM_TRACE=1
_TRACE = os.environ.get("SPMM_TRACE", "0") == "1"


def _install_ntff_hook():
    """Optional NTFF profiling hook for axon (used only when SPMM_TRACE=1)."""
    import contextlib
    import ctypes
    import sys
    import types

    try:
        import antenv.axon_hooks  # noqa: F401
        return
    except ImportError:
        pass
    so_path = "/opt/axon/libaxon_pjrt.so"
    if not os.path.exists(so_path):
        return
    lib = ctypes.CDLL(so_path)
    if not hasattr(lib, "axon_start_nrt_profile"):
        return
    lib.axon_start_nrt_profile.argtypes = [ctypes.POINTER(ctypes.c_int64),
                                           ctypes.c_size_t]
    lib.axon_start_nrt_profile.restype = ctypes.c_int64
    lib.axon_stop_nrt_profile.argtypes = [ctypes.c_char_p]
    lib.axon_stop_nrt_profile.restype = ctypes.c_int64

    @contextlib.contextmanager
    def _hook(output_dir, device_ids):
        import jax
        jax.devices()
        if device_ids:
            ids = (ctypes.c_int64 * len(device_ids))(*device_ids)
            rc = lib.axon_start_nrt_profile(ids, len(device_ids))
        else:
            rc = lib.axon_start_nrt_profile(None, 0)
        if rc != 0:
            raise RuntimeError(f"axon_start_nrt_profile rc={rc}")
        try:
            yield
        finally:
            lib.axon_stop_nrt_profile(str(output_dir).encode())

    mod = types.ModuleType("antenv.axon_hooks")
    mod.get_axon_ntff_profile_hook = lambda: _hook
    mod.set_axon_ntff_profile_hook = lambda h: None
    import antenv
    antenv.axon_hooks = mod
    sys.modules["antenv.axon_hooks"] = mod


# ---------------------------------------------------------------- planning
def build_plan(dst_idx, src_idx, weights):
    deg = np.bincount(dst_idx, minlength=OUT_SIZE)
    order = np.argsort(-deg, kind="stable")
    snake = order.reshape(SHARD, NCORES)
    snake[1::2] = snake[1::2, ::-1]
    core_nodes = snake.T.copy()
    node_core = np.empty(OUT_SIZE, dtype=np.int64)
    node_local = np.empty(OUT_SIZE, dtype=np.int64)
    for k in range(NCORES):
        node_core[core_nodes[k]] = k
        node_local[core_nodes[k]] = np.arange(SHARD)

    e_core = node_core[dst_idx]
    e_local = node_local[dst_idx]
    e_half = src_idx // HALF

    groups = {}
    max_rounds = 0
    for k in range(NCORES):
        mk = e_core == k
        for h in range(NHALF):
            m = mk & (e_half == h)
            loc = e_local[m]
            srcl = (src_idx[m] - h * HALF).astype(np.int64)
            w = weights[m].astype(np.float32)
            cnt = np.bincount(loc, minlength=SHARD)
            perm = np.argsort(-cnt, kind="stable")
            rank = np.empty(SHARD, dtype=np.int64)
            rank[perm] = np.arange(SHARD)
            eorder = np.lexsort((np.arange(loc.size), rank[loc]))
            srcl_s = srcl[eorder]
            w_s = w[eorder]
            cnt_s = cnt[perm]
            starts = np.concatenate([[0], np.cumsum(cnt_s)[:-1]])
            nrounds = int(cnt_s[0]) if cnt_s.size else 0
            max_rounds = max(max_rounds, nrounds)
            groups[(k, h)] = dict(perm=perm, cnt_s=cnt_s, starts=starts,
                                  srcl=srcl_s, w=w_s, nrounds=nrounds)

    n_r = np.zeros(max_rounds, dtype=np.int64)
    for g in groups.values():
        cs = g["cnt_s"]
        nr = (cs[:, None] > np.arange(g["nrounds"])[None, :]).sum(axis=0)
        n_r[: nr.size] = np.maximum(n_r[: nr.size], nr)
    n_r_pad = ((n_r + RND_ALIGN - 1) // RND_ALIGN) * RND_ALIGN
    offs = np.concatenate([[0], np.cumsum(n_r_pad)]).astype(np.int64)
    L = int(offs[-1])

    idx_streams = np.zeros((NCORES, NHALF, L), dtype=np.int16)
    w_streams = np.zeros((NCORES, NHALF, L), dtype=np.float32)
    for (k, h), g in groups.items():
        cnt_s = g["cnt_s"]
        starts = g["starts"]
        for r in range(g["nrounds"]):
            n = int((cnt_s > r).sum())
            pos = starts[:n] + r
            o = offs[r]
            idx_streams[k, h, o:o + n] = g["srcl"][pos].astype(np.int16)
            w_streams[k, h, o:o + n] = g["w"][pos]

    sched = dict(L=L, n_r_pad=n_r_pad, offs=offs, rounds=max_rounds)
    meta = dict(core_nodes=core_nodes,
                perms={kh: g["perm"] for kh, g in groups.items()})
    return sched, idx_streams, w_streams, meta


def pack_core_inputs(sched, idx_streams, w_streams, k):
    L = sched["L"]
    idx_tile = np.zeros((128, NHALF * L // 16), dtype=np.int16)
    w_tile = np.empty((128, NHALF * L // 128), dtype=np.float32)
    for h in range(NHALF):
        wrap = idx_streams[k, h].reshape(-1, 16).T
        for g in range(8):
            idx_tile[16 * g:16 * g + 16, h * L // 16:(h + 1) * L // 16] = wrap
        w_tile[:, h * L // 128:(h + 1) * L // 128] = \
            w_streams[k, h].reshape(-1, 128).T
    return idx_tile, w_tile


def pack_x(x):
    xt = np.zeros((IN_SIZE, ELEM), dtype=np.float32)
    xt[:, :B] = x.T
    return xt


def host_post(acc_by_core, sched, meta):
    out = np.empty((B, OUT_SIZE), dtype=np.float32)
    for k in range(NCORES):
        shard_out = None
        for h in range(NHALF):
            acc = acc_by_core[k][h]                      # [128, 64, 32]
            flat = acc.transpose(1, 0, 2).reshape(SHARD, B)
            perm = meta["perms"][(k, h)]
            contrib = np.empty((B, SHARD), dtype=np.float32)
            contrib[:, perm] = flat.T
            shard_out = contrib if shard_out is None else shard_out + contrib
        out[:, meta["core_nodes"][k]] = shard_out
    return out


# ---------------------------------------------------------------- device
def _round_pieces(sched, chunk_blk_lo, chunk_blk_hi):
    offs = sched["offs"]
    n_r_pad = sched["n_r_pad"]
    for r in range(sched["rounds"]):
        lo = int(offs[r]) // 128
        hi = lo + int(n_r_pad[r]) // 128
        a = max(lo, chunk_blk_lo)
        b = min(hi, chunk_blk_hi)
        if a < b:
            yield r, a - lo, a - chunk_blk_lo, b - a


def build_nc(sched, n_gather_bufs=6):
    L = sched["L"]
    nchunk = -(-L // CHUNK_IDX)
    nc = bacc.Bacc("TRN2", target_bir_lowering=False, debug=False,
                   num_devices=NCORES, num_swdge_queues=4,
                   dynamic_dma_scratch_size=65536)
    xt_d = nc.dram_tensor("xt", [IN_SIZE, ELEM], mybir.dt.float32,
                          kind="ExternalInput")
    idx_d = nc.dram_tensor("idx", [128, NHALF * L // 16], mybir.dt.int16,
                           kind="ExternalInput")
    w_d = nc.dram_tensor("w", [128, NHALF * L // 128], mybir.dt.float32,
                         kind="ExternalInput")
    y_d = nc.dram_tensor("y", [128, NHALF * ACC_BLK * B], mybir.dt.float32,
                         kind="ExternalOutput")

    with TileContext(nc) as tc:
        with tc.tile_pool(name="meta", bufs=1) as mp, \
             tc.tile_pool(name="acc", bufs=1) as ap_, \
             tc.tile_pool(name="g", bufs=n_gather_bufs) as gp:
            it = mp.tile([128, NHALF * L // 16], mybir.dt.int16)
            wt = mp.tile([128, NHALF * L // 128], mybir.dt.float32)
            QI = 2048 // 16   # idx columns per 2048-idx piece
            QW = 2048 // 128
            npiece = -(-NHALF * L // 2048)
            for q in range(npiece):
                nc.sync.dma_start(out=it[:, q * QI:min((q + 1) * QI, NHALF * L // 16)],
                                  in_=idx_d[:, q * QI:min((q + 1) * QI, NHALF * L // 16)])
                nc.sync.dma_start(out=wt[:, q * QW:min((q + 1) * QW, NHALF * L // 128)],
                                  in_=w_d[:, q * QW:min((q + 1) * QW, NHALF * L // 128)])
            for h in range(NHALF):
                acc = ap_.tile([128, ACC_BLK, B], mybir.dt.float32,
                               tag=f"acc{h}")
                covered0 = int(sched["n_r_pad"][0]) // 128
                if covered0 < ACC_BLK:
                    nc.vector.memset(acc[:, covered0:, :], 0.0)
                for ci in range(nchunk):
                    i_lo = ci * CHUNK_IDX
                    i_hi = min(L, i_lo + CHUNK_IDX)
                    nidx = i_hi - i_lo
                    nblk = nidx // 128
                    gt = gp.tile([128, CHUNK_IDX // 128, ELEM],
                                 mybir.dt.float32)
                    nc.gpsimd.dma_gather(
                        gt[:, :nblk, :],
                        xt_d[h * HALF:(h + 1) * HALF, :],
                        it[:, (h * L + i_lo) // 16:(h * L + i_hi) // 16],
                        num_idxs=nidx, num_idxs_reg=nidx, elem_size=ELEM,
                        single_packet=True,
                        queue_num=(h * nchunk + ci) % 4)
                    w_b = wt[:, (h * L + i_lo) // 128:(h * L + i_hi) // 128,
                             None].broadcast_to([128, nblk, B])
                    nc.vector.tensor_tensor(
                        out=gt[:, :nblk, 0:B], in0=gt[:, :nblk, 0:B],
                        in1=w_b, op=mybir.AluOpType.mult)
                    for r, acc_off, c_off, nb in _round_pieces(
                            sched, i_lo // 128, i_hi // 128):
                        if r == 0:
                            nc.vector.tensor_copy(
                                out=acc[:, acc_off:acc_off + nb, :],
                                in_=gt[:, c_off:c_off + nb, 0:B])
                        else:
                            nc.vector.tensor_tensor(
                                out=acc[:, acc_off:acc_off + nb, :],
                                in0=acc[:, acc_off:acc_off + nb, :],
                                in1=gt[:, c_off:c_off + nb, 0:B],
                                op=mybir.AluOpType.add)
                nc.sync.dma_start(
                    out=y_d[:, h * ACC_BLK * B:(h + 1) * ACC_BLK * B],
                    in_=acc[:])
    nc.compile()
    return nc


# ---------------------------------------------------------------- kernel
def kernel(x, weights, dst_idx, src_idx):
    global LAST_EXEC_NS
    x = np.ascontiguousarray(np.asarray(x, dtype=np.float32))
    weights = np.ascontiguousarray(np.asarray(weights, dtype=np.float32))
    dst64 = np.asarray(dst_idx).astype(np.int64)
    src64 = np.asarray(src_idx).astype(np.int64)

    if _TRACE:
        _install_ntff_hook()

    sched, idx_streams, w_streams, meta = build_plan(dst64, src64, weights)
    xt = pack_x(x)
    in_maps = []
    for k in range(NCORES):
        idx_tile, w_tile = pack_core_inputs(sched, idx_streams, w_streams, k)
        in_maps.append({"xt": xt, "idx": idx_tile, "w": w_tile})

    nc = build_nc(sched)
    res = run_bass_kernel_spmd(nc, in_maps, core_ids=list(range(NCORES)),
                               trace=_TRACE)
    LAST_EXEC_NS = res.exec_time_ns

    acc_by_core = []
    for k in range(NCORES):
        y = res.results[k]["y"].reshape(128, NHALF, ACC_BLK, B)
        acc_by_core.append([y[:, h] for h in range(NHALF)])
    return host_post(acc_by_core, sched, meta).astype(np.float32)


# revision 11
# speedup vs baseline: 1.1989x; 1.1989x over previous
"""COO SpMM (out[b, dst] += w_e * x[b, src_e]) on 8 Trainium2 NeuronCores.

Shapes (hardcoded from the problem spec):
  x [32, 65536] f32, weights [1048576] f32, dst_idx/src_idx [1048576] i32
  out [32, 65536] f32

Strategy (SPMD over 8 cores, identical program, per-core data):
- Each core owns a dst shard of 8192 nodes (snake assignment by global dst
  degree, so per-core edge counts are balanced). No cross-core reduction.
- x.T is padded to [65536, 64] f32 rows (32 batch lanes + 32 pad = 256 B
  rows) and staged in HBM per core.
- Core edges are split by src half (int16 gather-index range); per
  (core, half) the local dst are sorted by half-degree descending; round r
  takes the r-th edge of every dst with >= r+1 edges in that half, so each
  round touches a prefix of the dst slots and each dst appears at most once
  per round (conflict-free accumulation).
- A global round schedule (max prefix length over all 16 (core, half)
  groups, padded to 128) keeps the device program identical across cores;
  padding slots gather row 0 with weight 0.
- Device per core: gpsimd.dma_gather streams 256 B x.T rows from HBM into
  SBUF [128, blk, 64] (stream pos i -> partition i%128, block i//128);
  DVE multiplies in-place by the per-row weight (stride-0 broadcast along
  the 32 batch lanes); round 0 is a copy and rounds r>=1 are adds into
  acc_h [128, 64, 32]; both half-accumulators are DMAed out.
- Host: per (core, half) un-permute the degree-sort, sum halves, place the
  shard columns into the full [32, 65536] output.
"""
import os

import numpy as np

import concourse.bacc as bacc
import concourse.mybir as mybir
from concourse.bass_utils import run_bass_kernel_spmd
from concourse.tile import TileContext

B = 32
IN_SIZE = 65536
OUT_SIZE = 65536
NCORES = 8
SHARD = OUT_SIZE // NCORES          # 8192
NHALF = 2
HALF = IN_SIZE // NHALF             # 32768
ELEM = 64                           # padded x.T row, f32 units (256 B)
RND_ALIGN = 128
CHUNK_IDX = 8192                    # idxs per dma_gather call
ACC_BLK = SHARD // 128              # 64

LAST_EXEC_NS = None                 # set when SPMM_TRACE=1
_TRACE = os.environ.get("SPMM_TRACE", "0") == "1"


def _install_ntff_hook():
    """Optional NTFF profiling hook for axon (used only when SPMM_TRACE=1)."""
    import contextlib
    import ctypes
    import sys
    import types

    try:
        import antenv.axon_hooks  # noqa: F401
        return
    except ImportError:
        pass
    so_path = "/opt/axon/libaxon_pjrt.so"
    if not os.path.exists(so_path):
        return
    lib = ctypes.CDLL(so_path)
    if not hasattr(lib, "axon_start_nrt_profile"):
        return
    lib.axon_start_nrt_profile.argtypes = [ctypes.POINTER(ctypes.c_int64),
                                           ctypes.c_size_t]
    lib.axon_start_nrt_profile.restype = ctypes.c_int64
    lib.axon_stop_nrt_profile.argtypes = [ctypes.c_char_p]
    lib.axon_stop_nrt_profile.restype = ctypes.c_int64

    @contextlib.contextmanager
    def _hook(output_dir, device_ids):
        import jax
        jax.devices()
        if device_ids:
            ids = (ctypes.c_int64 * len(device_ids))(*device_ids)
            rc = lib.axon_start_nrt_profile(ids, len(device_ids))
        else:
            rc = lib.axon_start_nrt_profile(None, 0)
        if rc != 0:
            raise RuntimeError(f"axon_start_nrt_profile rc={rc}")
        try:
            yield
        finally:
            lib.axon_stop_nrt_profile(str(output_dir).encode())

    mod = types.ModuleType("antenv.axon_hooks")
    mod.get_axon_ntff_profile_hook = lambda: _hook
    mod.set_axon_ntff_profile_hook = lambda h: None
    import antenv
    antenv.axon_hooks = mod
    sys.modules["antenv.axon_hooks"] = mod


# ---------------------------------------------------------------- planning
def build_plan(dst_idx, src_idx, weights):
    deg = np.bincount(dst_idx, minlength=OUT_SIZE)
    order = np.argsort(-deg, kind="stable")
    snake = order.reshape(SHARD, NCORES)
    snake[1::2] = snake[1::2, ::-1]
    core_nodes = snake.T.copy()
    node_core = np.empty(OUT_SIZE, dtype=np.int64)
    node_local = np.empty(OUT_SIZE, dtype=np.int64)
    for k in range(NCORES):
        node_core[core_nodes[k]] = k
        node_local[core_nodes[k]] = np.arange(SHARD)

    e_core = node_core[dst_idx]
    e_local = node_local[dst_idx]
    e_half = src_idx // HALF

    groups = {}
    max_rounds = 0
    for k in range(NCORES):
        mk = e_core == k
        for h in range(NHALF):
            m = mk & (e_half == h)
            loc = e_local[m]
            srcl = (src_idx[m] - h * HALF).astype(np.int64)
            w = weights[m].astype(np.float32)
            cnt = np.bincount(loc, minlength=SHARD)
            perm = np.argsort(-cnt, kind="stable")
            rank = np.empty(SHARD, dtype=np.int64)
            rank[perm] = np.arange(SHARD)
            eorder = np.lexsort((np.arange(loc.size), rank[loc]))
            srcl_s = srcl[eorder]
            w_s = w[eorder]
            cnt_s = cnt[perm]
            starts = np.concatenate([[0], np.cumsum(cnt_s)[:-1]])
            nrounds = int(cnt_s[0]) if cnt_s.size else 0
            max_rounds = max(max_rounds, nrounds)
            groups[(k, h)] = dict(perm=perm, cnt_s=cnt_s, starts=starts,
                                  srcl=srcl_s, w=w_s, nrounds=nrounds)

    n_r = np.zeros(max_rounds, dtype=np.int64)
    for g in groups.values():
        cs = g["cnt_s"]
        nr = (cs[:, None] > np.arange(g["nrounds"])[None, :]).sum(axis=0)
        n_r[: nr.size] = np.maximum(n_r[: nr.size], nr)
    n_r_pad = ((n_r + RND_ALIGN - 1) // RND_ALIGN) * RND_ALIGN
    offs = np.concatenate([[0], np.cumsum(n_r_pad)]).astype(np.int64)
    L = int(offs[-1])

    idx_streams = np.zeros((NCORES, NHALF, L), dtype=np.int16)
    w_streams = np.zeros((NCORES, NHALF, L), dtype=np.float32)
    for (k, h), g in groups.items():
        cnt_s = g["cnt_s"]
        starts = g["starts"]
        for r in range(g["nrounds"]):
            n = int((cnt_s > r).sum())
            pos = starts[:n] + r
            o = offs[r]
            idx_streams[k, h, o:o + n] = g["srcl"][pos].astype(np.int16)
            w_streams[k, h, o:o + n] = g["w"][pos]

    sched = dict(L=L, n_r_pad=n_r_pad, offs=offs, rounds=max_rounds)
    meta = dict(core_nodes=core_nodes,
                perms={kh: g["perm"] for kh, g in groups.items()})
    return sched, idx_streams, w_streams, meta


def pack_core_inputs(sched, idx_streams, w_streams, k):
    L = sched["L"]
    idx_tile = np.zeros((128, NHALF * L // 16), dtype=np.int16)
    w_tile = np.empty((128, NHALF * L // 128), dtype=np.float32)
    for h in range(NHALF):
        wrap = idx_streams[k, h].reshape(-1, 16).T
        for g in range(8):
            idx_tile[16 * g:16 * g + 16, h * L // 16:(h + 1) * L // 16] = wrap
        w_tile[:, h * L // 128:(h + 1) * L // 128] = \
            w_streams[k, h].reshape(-1, 128).T
    return idx_tile, w_tile


def pack_x(x):
    xt = np.zeros((IN_SIZE, ELEM), dtype=np.float32)
    xt[:, :B] = x.T
    return xt


def host_post(acc_by_core, sched, meta):
    out = np.empty((B, OUT_SIZE), dtype=np.float32)
    for k in range(NCORES):
        shard_out = None
        for h in range(NHALF):
            acc = acc_by_core[k][h]                      # [128, 64, 32]
            flat = acc.transpose(1, 0, 2).reshape(SHARD, B)
            perm = meta["perms"][(k, h)]
            contrib = np.empty((B, SHARD), dtype=np.float32)
            contrib[:, perm] = flat.T
            shard_out = contrib if shard_out is None else shard_out + contrib
        out[:, meta["core_nodes"][k]] = shard_out
    return out


# ---------------------------------------------------------------- device
def _round_pieces(sched, chunk_blk_lo, chunk_blk_hi):
    offs = sched["offs"]
    n_r_pad = sched["n_r_pad"]
    for r in range(sched["rounds"]):
        lo = int(offs[r]) // 128
        hi = lo + int(n_r_pad[r]) // 128
        a = max(lo, chunk_blk_lo)
        b = min(hi, chunk_blk_hi)
        if a < b:
            yield r, a - lo, a - chunk_blk_lo, b - a


def build_nc(sched, n_gather_bufs=4):
    L = sched["L"]
    nchunk = -(-L // CHUNK_IDX)
    nc = bacc.Bacc("TRN2", target_bir_lowering=False, debug=False,
                   num_devices=NCORES, num_swdge_queues=4,
                   dynamic_dma_scratch_size=65536)
    xt_d = nc.dram_tensor("xt", [IN_SIZE, ELEM], mybir.dt.float32,
                          kind="ExternalInput")
    idx_d = nc.dram_tensor("idx", [128, NHALF * L // 16], mybir.dt.int16,
                           kind="ExternalInput")
    w_d = nc.dram_tensor("w", [128, NHALF * L // 128], mybir.dt.float32,
                         kind="ExternalInput")
    y_d = nc.dram_tensor("y", [128, NHALF * ACC_BLK * B], mybir.dt.float32,
                         kind="ExternalOutput")

    with TileContext(nc) as tc:
        with tc.tile_pool(name="meta", bufs=1) as mp, \
             tc.tile_pool(name="acc", bufs=1) as ap_, \
             tc.tile_pool(name="g", bufs=n_gather_bufs) as gp:
            it = mp.tile([128, NHALF * L // 16], mybir.dt.int16)
            wt = mp.tile([128, NHALF * L // 128], mybir.dt.float32)
            QI = 2048 // 16   # idx columns per 2048-idx piece
            QW = 2048 // 128
            npiece = -(-NHALF * L // 2048)
            for q in range(npiece):
                nc.sync.dma_start(out=it[:, q * QI:min((q + 1) * QI, NHALF * L // 16)],
                                  in_=idx_d[:, q * QI:min((q + 1) * QI, NHALF * L // 16)])
                nc.sync.dma_start(out=wt[:, q * QW:min((q + 1) * QW, NHALF * L // 128)],
                                  in_=w_d[:, q * QW:min((q + 1) * QW, NHALF * L // 128)])
            for h in range(NHALF):
                acc = ap_.tile([128, ACC_BLK, B], mybir.dt.float32,
                               tag=f"acc{h}")
                covered0 = int(sched["n_r_pad"][0]) // 128
                if covered0 < ACC_BLK:
                    nc.vector.memset(acc[:, covered0:, :], 0.0)
                for ci in range(nchunk):
                    i_lo = ci * CHUNK_IDX
                    i_hi = min(L, i_lo + CHUNK_IDX)
                    nidx = i_hi - i_lo
                    nblk = nidx // 128
                    gt = gp.tile([128, CHUNK_IDX // 128, ELEM],
                                 mybir.dt.float32)
                    nc.gpsimd.dma_gather(
                        gt[:, :nblk, :],
                        xt_d[h * HALF:(h + 1) * HALF, :],
                        it[:, (h * L + i_lo) // 16:(h * L + i_hi) // 16],
                        num_idxs=nidx, num_idxs_reg=nidx, elem_size=ELEM,
                        single_packet=False,
                        queue_num=(h * nchunk + ci) % 4)
                    w_b = wt[:, (h * L + i_lo) // 128:(h * L + i_hi) // 128,
                             None].broadcast_to([128, nblk, B])
                    nc.vector.tensor_tensor(
                        out=gt[:, :nblk, 0:B], in0=gt[:, :nblk, 0:B],
                        in1=w_b, op=mybir.AluOpType.mult)
                    for r, acc_off, c_off, nb in _round_pieces(
                            sched, i_lo // 128, i_hi // 128):
                        if r == 0:
                            nc.vector.tensor_copy(
                                out=acc[:, acc_off:acc_off + nb, :],
                                in_=gt[:, c_off:c_off + nb, 0:B])
                        else:
                            nc.vector.tensor_tensor(
                                out=acc[:, acc_off:acc_off + nb, :],
                                in0=acc[:, acc_off:acc_off + nb, :],
                                in1=gt[:, c_off:c_off + nb, 0:B],
                                op=mybir.AluOpType.add)
                nc.sync.dma_start(
                    out=y_d[:, h * ACC_BLK * B:(h + 1) * ACC_BLK * B],
                    in_=acc[:])
    nc.compile()
    return nc


# ---------------------------------------------------------------- kernel
def kernel(x, weights, dst_idx, src_idx):
    global LAST_EXEC_NS
    x = np.ascontiguousarray(np.asarray(x, dtype=np.float32))
    weights = np.ascontiguousarray(np.asarray(weights, dtype=np.float32))
    dst64 = np.asarray(dst_idx).astype(np.int64)
    src64 = np.asarray(src_idx).astype(np.int64)

    if _TRACE:
        _install_ntff_hook()

    sched, idx_streams, w_streams, meta = build_plan(dst64, src64, weights)
    xt = pack_x(x)
    in_maps = []
    for k in range(NCORES):
        idx_tile, w_tile = pack_core_inputs(sched, idx_streams, w_streams, k)
        in_maps.append({"xt": xt, "idx": idx_tile, "w": w_tile})

    nc = build_nc(sched)
    res = run_bass_kernel_spmd(nc, in_maps, core_ids=list(range(NCORES)),
                               trace=_TRACE)
    LAST_EXEC_NS = res.exec_time_ns

    acc_by_core = []
    for k in range(NCORES):
        y = res.results[k]["y"].reshape(128, NHALF, ACC_BLK, B)
        acc_by_core.append([y[:, h] for h in range(NHALF)])
    return host_post(acc_by_core, sched, meta).astype(np.float32)


# revision 13
# speedup vs baseline: 2.0852x; 1.7393x over previous
"""COO SpMM (out[b, dst] += w_e * x[b, src_e]) on 8 Trainium2 NeuronCores.

Shapes (hardcoded from the problem spec):
  x [32, 65536] f32, weights [1048576] f32, dst_idx/src_idx [1048576] i32
  out [32, 65536] f32

Strategy (SPMD over 8 cores, identical program, per-core data):
- Each core owns a dst shard of 8192 nodes (snake assignment by global dst
  degree, so per-core edge counts are balanced). No cross-core reduction.
- x.T is padded to [65536, 64] f32 rows (32 batch lanes + 32 pad = 256 B
  rows) and staged in HBM per core.
- Core edges are split by src half (int16 gather-index range); per
  (core, half) the local dst are sorted by half-degree descending; round r
  takes the r-th edge of every dst with >= r+1 edges in that half, so each
  round touches a prefix of the dst slots and each dst appears at most once
  per round (conflict-free accumulation).
- A global round schedule (max prefix length over all 16 (core, half)
  groups, padded to 128) keeps the device program identical across cores;
  padding slots gather row 0 with weight 0.
- Device per core: gpsimd.dma_gather streams 256 B x.T rows from HBM into
  SBUF [128, blk, 64] (stream pos i -> partition i%128, block i//128);
  DVE multiplies in-place by the per-row weight (stride-0 broadcast along
  the 32 batch lanes); round 0 is a copy and rounds r>=1 are adds into
  acc_h [128, 64, 32]; both half-accumulators are DMAed out.
- Host: per (core, half) un-permute the degree-sort, sum halves, place the
  shard columns into the full [32, 65536] output.
"""
import os

import numpy as np

import concourse.bacc as bacc
import concourse.mybir as mybir
from concourse.bass_utils import run_bass_kernel_spmd
from concourse.tile import TileContext

B = 32
IN_SIZE = 65536
OUT_SIZE = 65536
NCORES = 8
SHARD = OUT_SIZE // NCORES          # 8192
NHALF = 2
HALF = IN_SIZE // NHALF             # 32768
ELEM = 64                           # padded x.T row, f32 units (256 B)
RND_ALIGN = 128
CHUNK_IDX = 1024                    # idxs per dma_gather call
ACC_BLK = SHARD // 128              # 64

LAST_EXEC_NS = None                 # set when SPMM_TRACE=1
_TRACE = os.environ.get("SPMM_TRACE", "0") == "1"


def _install_ntff_hook():
    """Optional NTFF profiling hook for axon (used only when SPMM_TRACE=1)."""
    import contextlib
    import ctypes
    import sys
    import types

    try:
        import antenv.axon_hooks  # noqa: F401
        return
    except ImportError:
        pass
    so_path = "/opt/axon/libaxon_pjrt.so"
    if not os.path.exists(so_path):
        return
    lib = ctypes.CDLL(so_path)
    if not hasattr(lib, "axon_start_nrt_profile"):
        return
    lib.axon_start_nrt_profile.argtypes = [ctypes.POINTER(ctypes.c_int64),
                                           ctypes.c_size_t]
    lib.axon_start_nrt_profile.restype = ctypes.c_int64
    lib.axon_stop_nrt_profile.argtypes = [ctypes.c_char_p]
    lib.axon_stop_nrt_profile.restype = ctypes.c_int64

    @contextlib.contextmanager
    def _hook(output_dir, device_ids):
        import jax
        jax.devices()
        if device_ids:
            ids = (ctypes.c_int64 * len(device_ids))(*device_ids)
            rc = lib.axon_start_nrt_profile(ids, len(device_ids))
        else:
            rc = lib.axon_start_nrt_profile(None, 0)
        if rc != 0:
            raise RuntimeError(f"axon_start_nrt_profile rc={rc}")
        try:
            yield
        finally:
            lib.axon_stop_nrt_profile(str(output_dir).encode())

    mod = types.ModuleType("antenv.axon_hooks")
    mod.get_axon_ntff_profile_hook = lambda: _hook
    mod.set_axon_ntff_profile_hook = lambda h: None
    import antenv
    antenv.axon_hooks = mod
    sys.modules["antenv.axon_hooks"] = mod


# ---------------------------------------------------------------- planning
def build_plan(dst_idx, src_idx, weights):
    deg = np.bincount(dst_idx, minlength=OUT_SIZE)
    order = np.argsort(-deg, kind="stable")
    snake = order.reshape(SHARD, NCORES)
    snake[1::2] = snake[1::2, ::-1]
    core_nodes = snake.T.copy()
    node_core = np.empty(OUT_SIZE, dtype=np.int64)
    node_local = np.empty(OUT_SIZE, dtype=np.int64)
    for k in range(NCORES):
        node_core[core_nodes[k]] = k
        node_local[core_nodes[k]] = np.arange(SHARD)

    e_core = node_core[dst_idx]
    e_local = node_local[dst_idx]
    e_half = src_idx // HALF

    groups = {}
    max_rounds = 0
    for k in range(NCORES):
        mk = e_core == k
        for h in range(NHALF):
            m = mk & (e_half == h)
            loc = e_local[m]
            srcl = (src_idx[m] - h * HALF).astype(np.int64)
            w = weights[m].astype(np.float32)
            cnt = np.bincount(loc, minlength=SHARD)
            perm = np.argsort(-cnt, kind="stable")
            rank = np.empty(SHARD, dtype=np.int64)
            rank[perm] = np.arange(SHARD)
            eorder = np.lexsort((np.arange(loc.size), rank[loc]))
            srcl_s = srcl[eorder]
            w_s = w[eorder]
            cnt_s = cnt[perm]
            starts = np.concatenate([[0], np.cumsum(cnt_s)[:-1]])
            nrounds = int(cnt_s[0]) if cnt_s.size else 0
            max_rounds = max(max_rounds, nrounds)
            groups[(k, h)] = dict(perm=perm, cnt_s=cnt_s, starts=starts,
                                  srcl=srcl_s, w=w_s, nrounds=nrounds)

    n_r = np.zeros(max_rounds, dtype=np.int64)
    for g in groups.values():
        cs = g["cnt_s"]
        nr = (cs[:, None] > np.arange(g["nrounds"])[None, :]).sum(axis=0)
        n_r[: nr.size] = np.maximum(n_r[: nr.size], nr)
    n_r_pad = ((n_r + RND_ALIGN - 1) // RND_ALIGN) * RND_ALIGN
    offs = np.concatenate([[0], np.cumsum(n_r_pad)]).astype(np.int64)
    L = int(offs[-1])

    idx_streams = np.zeros((NCORES, NHALF, L), dtype=np.int16)
    w_streams = np.zeros((NCORES, NHALF, L), dtype=np.float32)
    for (k, h), g in groups.items():
        cnt_s = g["cnt_s"]
        starts = g["starts"]
        for r in range(g["nrounds"]):
            n = int((cnt_s > r).sum())
            pos = starts[:n] + r
            o = offs[r]
            idx_streams[k, h, o:o + n] = g["srcl"][pos].astype(np.int16)
            w_streams[k, h, o:o + n] = g["w"][pos]

    sched = dict(L=L, n_r_pad=n_r_pad, offs=offs, rounds=max_rounds)
    meta = dict(core_nodes=core_nodes,
                perms={kh: g["perm"] for kh, g in groups.items()})
    return sched, idx_streams, w_streams, meta


def pack_core_inputs(sched, idx_streams, w_streams, k):
    L = sched["L"]
    idx_tile = np.zeros((128, NHALF * L // 16), dtype=np.int16)
    w_tile = np.empty((128, NHALF * L // 128), dtype=np.float32)
    for h in range(NHALF):
        wrap = idx_streams[k, h].reshape(-1, 16).T
        for g in range(8):
            idx_tile[16 * g:16 * g + 16, h * L // 16:(h + 1) * L // 16] = wrap
        w_tile[:, h * L // 128:(h + 1) * L // 128] = \
            w_streams[k, h].reshape(-1, 128).T
    return idx_tile, w_tile


def pack_x(x):
    xt = np.zeros((IN_SIZE, ELEM), dtype=np.float32)
    xt[:, :B] = x.T
    return xt


def host_post(acc_by_core, sched, meta):
    out = np.empty((B, OUT_SIZE), dtype=np.float32)
    for k in range(NCORES):
        shard_out = None
        for h in range(NHALF):
            acc = acc_by_core[k][h]                      # [128, 64, 32]
            flat = acc.transpose(1, 0, 2).reshape(SHARD, B)
            perm = meta["perms"][(k, h)]
            contrib = np.empty((B, SHARD), dtype=np.float32)
            contrib[:, perm] = flat.T
            shard_out = contrib if shard_out is None else shard_out + contrib
        out[:, meta["core_nodes"][k]] = shard_out
    return out


# ---------------------------------------------------------------- device
def _round_pieces(sched, chunk_blk_lo, chunk_blk_hi):
    offs = sched["offs"]
    n_r_pad = sched["n_r_pad"]
    for r in range(sched["rounds"]):
        lo = int(offs[r]) // 128
        hi = lo + int(n_r_pad[r]) // 128
        a = max(lo, chunk_blk_lo)
        b = min(hi, chunk_blk_hi)
        if a < b:
            yield r, a - lo, a - chunk_blk_lo, b - a


def build_nc(sched, n_gather_bufs=12):
    L = sched["L"]
    nchunk = -(-L // CHUNK_IDX)
    nc = bacc.Bacc("TRN2", target_bir_lowering=False, debug=False,
                   num_devices=NCORES, num_swdge_queues=4,
                   dynamic_dma_scratch_size=65536)
    xt_d = nc.dram_tensor("xt", [IN_SIZE, ELEM], mybir.dt.float32,
                          kind="ExternalInput")
    idx_d = nc.dram_tensor("idx", [128, NHALF * L // 16], mybir.dt.int16,
                           kind="ExternalInput")
    w_d = nc.dram_tensor("w", [128, NHALF * L // 128], mybir.dt.float32,
                         kind="ExternalInput")
    y_d = nc.dram_tensor("y", [128, NHALF * ACC_BLK * B], mybir.dt.float32,
                         kind="ExternalOutput")

    with TileContext(nc) as tc:
        with tc.tile_pool(name="meta", bufs=1) as mp, \
             tc.tile_pool(name="acc", bufs=1) as ap_, \
             tc.tile_pool(name="g", bufs=n_gather_bufs) as gp:
            it = mp.tile([128, NHALF * L // 16], mybir.dt.int16)
            wt = mp.tile([128, NHALF * L // 128], mybir.dt.float32)
            QI = 2048 // 16   # idx columns per 2048-idx piece
            QW = 2048 // 128
            npiece = -(-NHALF * L // 2048)
            for q in range(npiece):
                nc.sync.dma_start(out=it[:, q * QI:min((q + 1) * QI, NHALF * L // 16)],
                                  in_=idx_d[:, q * QI:min((q + 1) * QI, NHALF * L // 16)])
                nc.sync.dma_start(out=wt[:, q * QW:min((q + 1) * QW, NHALF * L // 128)],
                                  in_=w_d[:, q * QW:min((q + 1) * QW, NHALF * L // 128)])
            for h in range(NHALF):
                acc = ap_.tile([128, ACC_BLK, B], mybir.dt.float32,
                               tag=f"acc{h}")
                covered0 = int(sched["n_r_pad"][0]) // 128
                if covered0 < ACC_BLK:
                    nc.vector.memset(acc[:, covered0:, :], 0.0)
                for ci in range(nchunk):
                    i_lo = ci * CHUNK_IDX
                    i_hi = min(L, i_lo + CHUNK_IDX)
                    nidx = i_hi - i_lo
                    nblk = nidx // 128
                    gt = gp.tile([128, CHUNK_IDX // 128, ELEM],
                                 mybir.dt.float32)
                    nc.gpsimd.dma_gather(
                        gt[:, :nblk, :],
                        xt_d[h * HALF:(h + 1) * HALF, :],
                        it[:, (h * L + i_lo) // 16:(h * L + i_hi) // 16],
                        num_idxs=nidx, num_idxs_reg=nidx, elem_size=ELEM,
                        single_packet=True,
                        queue_num=(h * nchunk + ci) % 4)
                    w_b = wt[:, (h * L + i_lo) // 128:(h * L + i_hi) // 128,
                             None].broadcast_to([128, nblk, B])
                    nc.vector.tensor_tensor(
                        out=gt[:, :nblk, 0:B], in0=gt[:, :nblk, 0:B],
                        in1=w_b, op=mybir.AluOpType.mult)
                    for r, acc_off, c_off, nb in _round_pieces(
                            sched, i_lo // 128, i_hi // 128):
                        if r == 0:
                            nc.vector.tensor_copy(
                                out=acc[:, acc_off:acc_off + nb, :],
                                in_=gt[:, c_off:c_off + nb, 0:B])
                        else:
                            nc.vector.tensor_tensor(
                                out=acc[:, acc_off:acc_off + nb, :],
                                in0=acc[:, acc_off:acc_off + nb, :],
                                in1=gt[:, c_off:c_off + nb, 0:B],
                                op=mybir.AluOpType.add)
                nc.sync.dma_start(
                    out=y_d[:, h * ACC_BLK * B:(h + 1) * ACC_BLK * B],
                    in_=acc[:])
    nc.compile()
    return nc


# ---------------------------------------------------------------- kernel
def kernel(x, weights, dst_idx, src_idx):
    global LAST_EXEC_NS
    x = np.ascontiguousarray(np.asarray(x, dtype=np.float32))
    weights = np.ascontiguousarray(np.asarray(weights, dtype=np.float32))
    dst64 = np.asarray(dst_idx).astype(np.int64)
    src64 = np.asarray(src_idx).astype(np.int64)

    if _TRACE:
        _install_ntff_hook()

    sched, idx_streams, w_streams, meta = build_plan(dst64, src64, weights)
    xt = pack_x(x)
    in_maps = []
    for k in range(NCORES):
        idx_tile, w_tile = pack_core_inputs(sched, idx_streams, w_streams, k)
        in_maps.append({"xt": xt, "idx": idx_tile, "w": w_tile})

    nc = build_nc(sched)
    res = run_bass_kernel_spmd(nc, in_maps, core_ids=list(range(NCORES)),
                               trace=_TRACE)
    LAST_EXEC_NS = res.exec_time_ns

    acc_by_core = []
    for k in range(NCORES):
        y = res.results[k]["y"].reshape(128, NHALF, ACC_BLK, B)
        acc_by_core.append([y[:, h] for h in range(NHALF)])
    return host_post(acc_by_core, sched, meta).astype(np.float32)


# revision 14
# speedup vs baseline: 2.1309x; 1.0219x over previous
"""COO SpMM (out[b, dst] += w_e * x[b, src_e]) on 8 Trainium2 NeuronCores.

Shapes (hardcoded from the problem spec):
  x [32, 65536] f32, weights [1048576] f32, dst_idx/src_idx [1048576] i32
  out [32, 65536] f32

Strategy (SPMD over 8 cores, identical program, per-core data):
- Each core owns a dst shard of 8192 nodes (snake assignment by global dst
  degree, so per-core edge counts are balanced). No cross-core reduction.
- x.T is padded to [65536, 64] f32 rows (32 batch lanes + 32 pad = 256 B
  rows) and staged in HBM per core.
- Core edges are split by src half (int16 gather-index range); per
  (core, half) the local dst are sorted by half-degree descending; round r
  takes the r-th edge of every dst with >= r+1 edges in that half, so each
  round touches a prefix of the dst slots and each dst appears at most once
  per round (conflict-free accumulation).
- A global round schedule (max prefix length over all 16 (core, half)
  groups, padded to 128) keeps the device program identical across cores;
  padding slots gather row 0 with weight 0.
- Device per core: gpsimd.dma_gather streams 256 B x.T rows from HBM into
  SBUF [128, blk, 64] (stream pos i -> partition i%128, block i//128);
  DVE multiplies in-place by the per-row weight (stride-0 broadcast along
  the 32 batch lanes); round 0 is a copy and rounds r>=1 are adds into
  acc_h [128, 64, 32]; both half-accumulators are DMAed out.
- Host: per (core, half) un-permute the degree-sort, sum halves, place the
  shard columns into the full [32, 65536] output.
"""
import os

import numpy as np

import concourse.bacc as bacc
import concourse.mybir as mybir
from concourse.bass_utils import run_bass_kernel_spmd
from concourse.tile import TileContext

B = 32
IN_SIZE = 65536
OUT_SIZE = 65536
NCORES = 8
SHARD = OUT_SIZE // NCORES          # 8192
NHALF = 2
HALF = IN_SIZE // NHALF             # 32768
ELEM = 64                           # padded x.T row, f32 units (256 B)
RND_ALIGN = 128
CHUNK_IDX = 1024                    # idxs per dma_gather call
ACC_BLK = SHARD // 128              # 64

LAST_EXEC_NS = None                 # set when SPMM_TRACE=1
_TRACE = os.environ.get("SPMM_TRACE", "0") == "1"


def _install_ntff_hook():
    """Optional NTFF profiling hook for axon (used only when SPMM_TRACE=1)."""
    import contextlib
    import ctypes
    import sys
    import types

    try:
        import antenv.axon_hooks  # noqa: F401
        return
    except ImportError:
        pass
    so_path = "/opt/axon/libaxon_pjrt.so"
    if not os.path.exists(so_path):
        return
    lib = ctypes.CDLL(so_path)
    if not hasattr(lib, "axon_start_nrt_profile"):
        return
    lib.axon_start_nrt_profile.argtypes = [ctypes.POINTER(ctypes.c_int64),
                                           ctypes.c_size_t]
    lib.axon_start_nrt_profile.restype = ctypes.c_int64
    lib.axon_stop_nrt_profile.argtypes = [ctypes.c_char_p]
    lib.axon_stop_nrt_profile.restype = ctypes.c_int64

    @contextlib.contextmanager
    def _hook(output_dir, device_ids):
        import jax
        jax.devices()
        if device_ids:
            ids = (ctypes.c_int64 * len(device_ids))(*device_ids)
            rc = lib.axon_start_nrt_profile(ids, len(device_ids))
        else:
            rc = lib.axon_start_nrt_profile(None, 0)
        if rc != 0:
            raise RuntimeError(f"axon_start_nrt_profile rc={rc}")
        try:
            yield
        finally:
            lib.axon_stop_nrt_profile(str(output_dir).encode())

    mod = types.ModuleType("antenv.axon_hooks")
    mod.get_axon_ntff_profile_hook = lambda: _hook
    mod.set_axon_ntff_profile_hook = lambda h: None
    import antenv
    antenv.axon_hooks = mod
    sys.modules["antenv.axon_hooks"] = mod


# ---------------------------------------------------------------- planning
def build_plan(dst_idx, src_idx, weights):
    deg = np.bincount(dst_idx, minlength=OUT_SIZE)
    order = np.argsort(-deg, kind="stable")
    snake = order.reshape(SHARD, NCORES)
    snake[1::2] = snake[1::2, ::-1]
    core_nodes = snake.T.copy()
    node_core = np.empty(OUT_SIZE, dtype=np.int64)
    node_local = np.empty(OUT_SIZE, dtype=np.int64)
    for k in range(NCORES):
        node_core[core_nodes[k]] = k
        node_local[core_nodes[k]] = np.arange(SHARD)

    e_core = node_core[dst_idx]
    e_local = node_local[dst_idx]
    e_half = src_idx // HALF

    groups = {}
    max_rounds = 0
    for k in range(NCORES):
        mk = e_core == k
        for h in range(NHALF):
            m = mk & (e_half == h)
            loc = e_local[m]
            srcl = (src_idx[m] - h * HALF).astype(np.int64)
            w = weights[m].astype(np.float32)
            cnt = np.bincount(loc, minlength=SHARD)
            perm = np.argsort(-cnt, kind="stable")
            rank = np.empty(SHARD, dtype=np.int64)
            rank[perm] = np.arange(SHARD)
            eorder = np.lexsort((np.arange(loc.size), rank[loc]))
            srcl_s = srcl[eorder]
            w_s = w[eorder]
            cnt_s = cnt[perm]
            starts = np.concatenate([[0], np.cumsum(cnt_s)[:-1]])
            nrounds = int(cnt_s[0]) if cnt_s.size else 0
            max_rounds = max(max_rounds, nrounds)
            groups[(k, h)] = dict(perm=perm, cnt_s=cnt_s, starts=starts,
                                  srcl=srcl_s, w=w_s, nrounds=nrounds)

    n_r = np.zeros(max_rounds, dtype=np.int64)
    for g in groups.values():
        cs = g["cnt_s"]
        nr = (cs[:, None] > np.arange(g["nrounds"])[None, :]).sum(axis=0)
        n_r[: nr.size] = np.maximum(n_r[: nr.size], nr)
    n_r_pad = ((n_r + RND_ALIGN - 1) // RND_ALIGN) * RND_ALIGN
    offs = np.concatenate([[0], np.cumsum(n_r_pad)]).astype(np.int64)
    L = int(offs[-1])

    idx_streams = np.zeros((NCORES, NHALF, L), dtype=np.int16)
    w_streams = np.zeros((NCORES, NHALF, L), dtype=np.float32)
    for (k, h), g in groups.items():
        cnt_s = g["cnt_s"]
        starts = g["starts"]
        for r in range(g["nrounds"]):
            n = int((cnt_s > r).sum())
            pos = starts[:n] + r
            o = offs[r]
            idx_streams[k, h, o:o + n] = g["srcl"][pos].astype(np.int16)
            w_streams[k, h, o:o + n] = g["w"][pos]

    sched = dict(L=L, n_r_pad=n_r_pad, offs=offs, rounds=max_rounds)
    meta = dict(core_nodes=core_nodes,
                perms={kh: g["perm"] for kh, g in groups.items()})
    return sched, idx_streams, w_streams, meta


def pack_core_inputs(sched, idx_streams, w_streams, k):
    L = sched["L"]
    idx_tile = np.zeros((128, NHALF * L // 16), dtype=np.int16)
    w_tile = np.empty((128, NHALF * L // 128), dtype=np.float32)
    for h in range(NHALF):
        wrap = idx_streams[k, h].reshape(-1, 16).T
        for g in range(8):
            idx_tile[16 * g:16 * g + 16, h * L // 16:(h + 1) * L // 16] = wrap
        w_tile[:, h * L // 128:(h + 1) * L // 128] = \
            w_streams[k, h].reshape(-1, 128).T
    return idx_tile, w_tile


def pack_x(x):
    xt = np.zeros((IN_SIZE, ELEM), dtype=np.float32)
    xt[:, :B] = x.T
    return xt


def host_post(acc_by_core, sched, meta):
    out = np.empty((B, OUT_SIZE), dtype=np.float32)
    for k in range(NCORES):
        shard_out = None
        for h in range(NHALF):
            acc = acc_by_core[k][h]                      # [128, 64, 32]
            flat = acc.transpose(1, 0, 2).reshape(SHARD, B)
            perm = meta["perms"][(k, h)]
            contrib = np.empty((B, SHARD), dtype=np.float32)
            contrib[:, perm] = flat.T
            shard_out = contrib if shard_out is None else shard_out + contrib
        out[:, meta["core_nodes"][k]] = shard_out
    return out


# ---------------------------------------------------------------- device
def _round_pieces(sched, chunk_blk_lo, chunk_blk_hi):
    offs = sched["offs"]
    n_r_pad = sched["n_r_pad"]
    for r in range(sched["rounds"]):
        lo = int(offs[r]) // 128
        hi = lo + int(n_r_pad[r]) // 128
        a = max(lo, chunk_blk_lo)
        b = min(hi, chunk_blk_hi)
        if a < b:
            yield r, a - lo, a - chunk_blk_lo, b - a


def build_nc(sched, n_gather_bufs=16):
    L = sched["L"]
    nchunk = -(-L // CHUNK_IDX)
    nc = bacc.Bacc("TRN2", target_bir_lowering=False, debug=False,
                   num_devices=NCORES, num_swdge_queues=4,
                   dynamic_dma_scratch_size=65536)
    xt_d = nc.dram_tensor("xt", [IN_SIZE, ELEM], mybir.dt.float32,
                          kind="ExternalInput")
    idx_d = nc.dram_tensor("idx", [128, NHALF * L // 16], mybir.dt.int16,
                           kind="ExternalInput")
    w_d = nc.dram_tensor("w", [128, NHALF * L // 128], mybir.dt.float32,
                         kind="ExternalInput")
    y_d = nc.dram_tensor("y", [128, NHALF * ACC_BLK * B], mybir.dt.float32,
                         kind="ExternalOutput")

    with TileContext(nc) as tc:
        with tc.tile_pool(name="meta", bufs=1) as mp, \
             tc.tile_pool(name="acc", bufs=1) as ap_, \
             tc.tile_pool(name="g", bufs=n_gather_bufs) as gp:
            it = mp.tile([128, NHALF * L // 16], mybir.dt.int16)
            wt = mp.tile([128, NHALF * L // 128], mybir.dt.float32)
            QI = 2048 // 16   # idx columns per 2048-idx piece
            QW = 2048 // 128
            npiece = -(-NHALF * L // 2048)
            for q in range(npiece):
                nc.sync.dma_start(out=it[:, q * QI:min((q + 1) * QI, NHALF * L // 16)],
                                  in_=idx_d[:, q * QI:min((q + 1) * QI, NHALF * L // 16)])
                nc.sync.dma_start(out=wt[:, q * QW:min((q + 1) * QW, NHALF * L // 128)],
                                  in_=w_d[:, q * QW:min((q + 1) * QW, NHALF * L // 128)])
            for h in range(NHALF):
                acc = ap_.tile([128, ACC_BLK, B], mybir.dt.float32,
                               tag=f"acc{h}")
                covered0 = int(sched["n_r_pad"][0]) // 128
                if covered0 < ACC_BLK:
                    nc.vector.memset(acc[:, covered0:, :], 0.0)
                for ci in range(nchunk):
                    i_lo = ci * CHUNK_IDX
                    i_hi = min(L, i_lo + CHUNK_IDX)
                    nidx = i_hi - i_lo
                    nblk = nidx // 128
                    gt = gp.tile([128, CHUNK_IDX // 128, ELEM],
                                 mybir.dt.float32)
                    nc.gpsimd.dma_gather(
                        gt[:, :nblk, :],
                        xt_d[h * HALF:(h + 1) * HALF, :],
                        it[:, (h * L + i_lo) // 16:(h * L + i_hi) // 16],
                        num_idxs=nidx, num_idxs_reg=nidx, elem_size=ELEM,
                        single_packet=True,
                        queue_num=(h * nchunk + ci) % 4)
                    w_b = wt[:, (h * L + i_lo) // 128:(h * L + i_hi) // 128,
                             None].broadcast_to([128, nblk, B])
                    nc.vector.tensor_tensor(
                        out=gt[:, :nblk, 0:B], in0=gt[:, :nblk, 0:B],
                        in1=w_b, op=mybir.AluOpType.mult)
                    for r, acc_off, c_off, nb in _round_pieces(
                            sched, i_lo // 128, i_hi // 128):
                        if r == 0:
                            nc.vector.tensor_copy(
                                out=acc[:, acc_off:acc_off + nb, :],
                                in_=gt[:, c_off:c_off + nb, 0:B])
                        else:
                            nc.vector.tensor_tensor(
                                out=acc[:, acc_off:acc_off + nb, :],
                                in0=acc[:, acc_off:acc_off + nb, :],
                                in1=gt[:, c_off:c_off + nb, 0:B],
                                op=mybir.AluOpType.add)
                nc.sync.dma_start(
                    out=y_d[:, h * ACC_BLK * B:(h + 1) * ACC_BLK * B],
                    in_=acc[:])
    nc.compile()
    return nc


# ---------------------------------------------------------------- kernel
def kernel(x, weights, dst_idx, src_idx):
    global LAST_EXEC_NS
    x = np.ascontiguousarray(np.asarray(x, dtype=np.float32))
    weights = np.ascontiguousarray(np.asarray(weights, dtype=np.float32))
    dst64 = np.asarray(dst_idx).astype(np.int64)
    src64 = np.asarray(src_idx).astype(np.int64)

    if _TRACE:
        _install_ntff_hook()

    sched, idx_streams, w_streams, meta = build_plan(dst64, src64, weights)
    xt = pack_x(x)
    in_maps = []
    for k in range(NCORES):
        idx_tile, w_tile = pack_core_inputs(sched, idx_streams, w_streams, k)
        in_maps.append({"xt": xt, "idx": idx_tile, "w": w_tile})

    nc = build_nc(sched)
    res = run_bass_kernel_spmd(nc, in_maps, core_ids=list(range(NCORES)),
                               trace=_TRACE)
    LAST_EXEC_NS = res.exec_time_ns

    acc_by_core = []
    for k in range(NCORES):
        y = res.results[k]["y"].reshape(128, NHALF, ACC_BLK, B)
        acc_by_core.append([y[:, h] for h in range(NHALF)])
    return host_post(acc_by_core, sched, meta).astype(np.float32)
